# revision 16
# baseline (speedup 1.0000x reference)
"""Multi-head causal attention (B=1, S=2048, E=2048, H=16, DH=128) on 8 TRN2
NeuronCores.

Sharding: tensor-parallel over heads; core c owns heads 2c and 2c+1.
Pipeline, ordered so the serial AllGather chain starts as early as possible:
  S1: stream x^T (bf16) while computing Q^T/K^T head 0 (8 PSUM accumulators)
  S2: Q^T/K^T head 1
  S3: V s-blocks 0-7 (both heads)
  attn(0), attn(1)     -> their gathers fire at ~90-110us
  S4: V s-blocks 8-15
  attn(2), attn(3)
  tail: output projection per q-group (column-sharded: core c computes
        y[:, 256c:256(c+1)]); gathers for groups 0-2 are long done, only
        group 3's is partially exposed.
Host gathers by concatenating the 8 column slices.

attention g (both heads, sk-blocks in pairs, software-pipelined emission):
S^T = K @ Q^T, exp on ScalarE, block-causal mask as post-exp multiply,
denominators via a ones-column matmul, normalization via a K=1 broadcast
matmul + DVE multiply.

Precision: x^T and Wq/Wk/Wv bf16 (projection inputs only); Q^T/K^T/V and
all attention matmuls float32r (full PE rate, ~1.2e-4); O^T gather and
output projection bf16; all accumulation fp32 in PSUM.
"""
import os
import sys

if "/opt/trn_rl_repo" not in sys.path:
    sys.path.insert(0, "/opt/trn_rl_repo")

import numpy as np

B, S, E, H = 1, 2048, 2048, 16
DH = E // H          # 128
N_CORES = 8
HPC = H // N_CORES   # heads per core = 2
KT = E // 128        # 16 contraction tiles
QG = 512             # q-group width
NG = S // QG         # 4 q-groups
SBK = S // 128       # 16 s/sk blocks
CSL = E // N_CORES   # 256 output columns per core

_CACHE = {}


def _build(fp_name: str):
    import concourse.bass as bass  # noqa: F401
    import concourse.mybir as mybir
    import concourse.tile as tile
    from concourse import bacc

    FP = getattr(mybir.dt, fp_name)
    F32 = mybir.dt.float32
    BF16 = mybir.dt.bfloat16
    AF = mybir.ActivationFunctionType

    nc = bacc.Bacc("TRN2", target_bir_lowering=False, debug=False,
                   num_devices=N_CORES)

    xT_t = nc.dram_tensor("xT", [E, S], BF16, kind="ExternalInput")
    wq_t = nc.dram_tensor("wq", [128, KT * HPC * DH], BF16, kind="ExternalInput")
    wk_t = nc.dram_tensor("wk", [128, KT * HPC * DH], BF16, kind="ExternalInput")
    wv_t = nc.dram_tensor("wv", [128, KT * HPC * DH], BF16, kind="ExternalInput")
    bq_t = nc.dram_tensor("bq", [DH, HPC], F32, kind="ExternalInput")
    bk_t = nc.dram_tensor("bk", [DH, HPC], F32, kind="ExternalInput")
    bv_t = nc.dram_tensor("bv", [1, HPC * DH], F32, kind="ExternalInput")
    wo_t = nc.dram_tensor("wo", [128, KT * CSL], BF16, kind="ExternalInput")
    bo_t = nc.dram_tensor("bo", [1, CSL], F32, kind="ExternalInput")
    mask_t = nc.dram_tensor("mask", [4 * 128, QG], BF16, kind="ExternalInput")
    y_t = nc.dram_tensor("y", [S, CSL], F32, kind="ExternalOutput")

    xT_r = xT_t.ap().rearrange("(kt p) s -> kt p s", p=128)
    mask_r = mask_t.ap().rearrange("(jm p) q -> jm p q", p=128)

    scale = 1.0 / float(np.sqrt(DH))

    with tile.TileContext(nc) as tc:
        with tc.tile_pool(name="const", bufs=1) as constp, \
             tc.tile_pool(name="prod", bufs=1) as prodp, \
             tc.tile_pool(name="dram", bufs=1, space="DRAM") as dramp:
            # --- constants ---
            ones_f32 = constp.tile([128, 128], F32)
            nc.vector.memset(ones_f32[:], 1.0)
            ones_col = constp.tile([128, 1], FP)
            nc.vector.tensor_copy(ones_col[:], ones_f32[:, 0:1])
            ones_row = constp.tile([1, 128], FP)
            nc.vector.tensor_copy(ones_row[:], ones_f32[0:1, :])
            bqs = constp.tile([DH, HPC], F32)
            nc.sync.dma_start(bqs[:], bq_t.ap()[:])
            bks = constp.tile([DH, HPC], F32)
            nc.sync.dma_start(bks[:], bk_t.ap()[:])
            bvs = constp.tile([128, HPC * DH], F32)
            nc.sync.dma_start(bvs[:], bv_t.ap().to_broadcast((128, HPC * DH)))
            bos = constp.tile([128, CSL], F32)
            nc.sync.dma_start(bos[:], bo_t.ap().to_broadcast((128, CSL)))
            masks = constp.tile([128, 4 * QG], BF16)
            for jm in range(4):
                nc.sync.dma_start(masks[:, jm * QG:(jm + 1) * QG], mask_r[jm])

            # --- products ---
            qkt = prodp.tile([128, HPC * S], FP)   # Q^T, head hh at cols hh*S
            kkt = prodp.tile([128, HPC * S], FP)   # K^T
            vt = prodp.tile([128, SBK * HPC * DH], FP)  # V, s-block sb at sb*256

            cin = [dramp.tile([HPC * DH, QG], BF16, tag=f"cin{g}",
                              name=f"cin{g}") for g in range(NG)]
            cout = [dramp.tile([N_CORES, HPC * DH, QG], BF16,
                               tag=f"cout{g}", name=f"cout{g}",
                               addr_space="Shared") for g in range(NG)]

            wsb = {}
            for nm_, t_ in (("wq", wq_t), ("wk", wk_t), ("wv", wv_t)):
                wtile_all = constp.tile([128, KT * HPC * DH], BF16,
                                        tag=f"wsb_{nm_}", name=f"wsb_{nm_}")
                nc.sync.dma_start(wtile_all[:], t_.ap()[:])
                wsb[nm_] = wtile_all

            with tc.tile_pool(name="xt", bufs=1) as xtp, \
                 tc.tile_pool(name="pt", bufs=6) as ptp, \
                 tc.tile_pool(name="osb", bufs=1) as osbp, \
                 tc.tile_pool(name="rec", bufs=2) as recp, \
                 tc.tile_pool(name="bcs", bufs=2) as bcsp:
                xt = xtp.tile([128, KT * S], BF16)
                o_sbuf = osbp.tile([128, HPC * S], BF16)

                def qk_pass(hh, stream):
                    """Q^T and K^T for head hh, all 4 q-groups: 8 PSUM accs."""
                    with tc.tile_pool(name=f"psQK{hh}", bufs=8,
                                      space="PSUM") as psA:
                        specs = [("wq", qkt, bqs), ("wk", kkt, bks)]
                        accs = {p: [psA.tile([128, QG], F32, tag="qk",
                                             name=f"qk{hh}_{p}{g}")
                                    for g in range(NG)] for p in range(2)}
                        for kt in range(KT):
                            if stream:
                                nc.sync.dma_start(xt[:, kt * S:(kt + 1) * S],
                                                  xT_r[kt])
                            for p, (wn, prod, bias) in enumerate(specs):
                                wtile = wsb[wn][:, kt * HPC * DH + hh * DH:
                                                kt * HPC * DH + (hh + 1) * DH]
                                for g in range(NG):
                                    nc.tensor.matmul(
                                        accs[p][g][:], wtile,
                                        xt[:, kt * S + g * QG:
                                           kt * S + (g + 1) * QG],
                                        start=(kt == 0), stop=(kt == KT - 1))
                        for p, (wn, prod, bias) in enumerate(specs):
                            for g in range(NG):
                                nc.scalar.activation(
                                    prod[:, hh * S + g * QG:
                                         hh * S + (g + 1) * QG],
                                    accs[p][g][:], AF.Identity,
                                    bias=bias[:, hh:hh + 1])

                def v_pass(half):
                    """V s-blocks half*8 .. half*8+7, both heads."""
                    with tc.tile_pool(name=f"psV{half}", bufs=8,
                                      space="PSUM") as psV:
                        accs = [psV.tile([128, HPC * DH], F32, tag="v",
                                         name=f"v{half}_{i}")
                                for i in range(8)]
                        for kt in range(KT):
                            wvtile = wsb["wv"][:, kt * HPC * DH:
                                               (kt + 1) * HPC * DH]
                            for i in range(8):
                                sb = half * 8 + i
                                nc.tensor.matmul(
                                    accs[i][:],
                                    xt[:, kt * S + sb * 128:
                                       kt * S + (sb + 1) * 128],
                                    wvtile,
                                    start=(kt == 0), stop=(kt == KT - 1))
                        for i in range(8):
                            sb = half * 8 + i
                            nc.vector.tensor_add(
                                vt[:, sb * HPC * DH:(sb + 1) * HPC * DH],
                                accs[i][:], bvs[:])

                def attn(g):
                    npairs = 2 * g + 2
                    jmax = 4 * g + 3
                    with tc.tile_pool(name=f"psS{g}", bufs=2,
                                      space="PSUM") as psS, \
                         tc.tile_pool(name=f"psO{g}", bufs=2,
                                      space="PSUM") as psO, \
                         tc.tile_pool(name=f"psN{g}", bufs=2,
                                      space="PSUM") as psN:
                        o_acc = [psO.tile([128, QG], F32, tag="o",
                                          name=f"o{hh}") for hh in range(HPC)]
                        s_acc = [psN.tile([1, QG], F32, tag="n",
                                          name=f"n{hh}") for hh in range(HPC)]

                        def emit_pv(hh, jp, pt):
                            for dj in range(2):
                                j = 2 * jp + dj
                                nc.tensor.matmul(
                                    o_acc[hh][:],
                                    vt[:, j * HPC * DH + hh * DH:
                                       j * HPC * DH + (hh + 1) * DH],
                                    pt[:, dj * QG:(dj + 1) * QG],
                                    start=(j == 0), stop=(j == jmax))
                                nc.tensor.matmul(
                                    s_acc[hh][:], ones_col[:],
                                    pt[:, dj * QG:(dj + 1) * QG],
                                    start=(j == 0), stop=(j == jmax))

                        pend = []
                        for jp in range(npairs):
                            for hh in range(HPC):
                                ps = psS.tile([128, 2 * QG], F32, tag="s",
                                              name="ps")
                                for dj in range(2):
                                    j = 2 * jp + dj
                                    nc.tensor.matmul(
                                        ps[:, dj * QG:(dj + 1) * QG],
                                        kkt[:, hh * S + j * 128:
                                            hh * S + (j + 1) * 128],
                                        qkt[:, hh * S + g * QG:
                                            hh * S + (g + 1) * QG],
                                        start=True, stop=True)
                                pt = ptp.tile([128, 2 * QG], FP, tag="p",
                                              name="pt")
                                nc.scalar.activation(pt[:], ps[:], AF.Exp,
                                                     scale=scale)
                                if 2 * jp >= 4 * g:
                                    jms = 2 * jp - 4 * g
                                    nc.vector.tensor_mul(
                                        pt[:], pt[:],
                                        masks[:, jms * QG:(jms + 2) * QG])
                                pend.append((hh, jp, pt))
                                while len(pend) > 2:
                                    emit_pv(*pend.pop(0))
                        while pend:
                            emit_pv(*pend.pop(0))

                        for hh in range(HPC):
                            rec = recp.tile([1, QG], FP, tag="r", name="rec")
                            with nc.allow_low_precision(
                                    reason="softmax denom recip in fp32r"):
                                nc.vector.reciprocal(rec[:], s_acc[hh][:])
                            bc = psS.tile([128, QG], F32, tag="s", name="bc")
                            nc.tensor.matmul(bc[:], ones_row[:], rec[:],
                                             start=True, stop=True)
                            bcs = bcsp.tile([128, QG], F32, tag="b",
                                            name="bcs")
                            nc.vector.tensor_copy(bcs[:], bc[:])
                            nc.vector.tensor_mul(
                                o_sbuf[:, hh * S + g * QG:
                                       hh * S + (g + 1) * QG],
                                o_acc[hh][:], bcs[:])
                            nc.sync.dma_start(
                                cin[g].rearrange("(hh p) q -> hh p q",
                                                 p=128)[hh],
                                o_sbuf[:, hh * S + g * QG:
                                       hh * S + (g + 1) * QG])
                    nc.gpsimd.collective_compute(
                        "AllGather",
                        mybir.AluOpType.bypass,
                        replica_groups=[list(range(N_CORES))],
                        ins=[cin[g].opt()],
                        outs=[cout[g].opt()],
                    )

                qk_pass(0, stream=True)
                qk_pass(1, stream=False)
                v_pass(0)
                attn(0)
                attn(1)
                v_pass(1)
                attn(2)
                attn(3)

            # ===== tail: output projection, column-sharded =====
            with tc.tile_pool(name="wo", bufs=1) as wop, \
                 tc.tile_pool(name="og", bufs=2) as ogp, \
                 tc.tile_pool(name="yst", bufs=2) as ystp, \
                 tc.tile_pool(name="psY", bufs=4, space="PSUM") as psY:
                wos = wop.tile([128, KT * CSL], BF16)
                nc.sync.dma_start(wos[:], wo_t.ap()[:])
                for g in range(NG):
                    og = ogp.tile([128, KT * QG], BF16, tag="og", name="og")
                    for kt in range(KT):
                        nc.sync.dma_start(
                            og[:, kt * QG:(kt + 1) * QG],
                            cout[g][kt // HPC, (kt % HPC) * 128:
                                    (kt % HPC) * 128 + 128, :])
                    for i in range(4):
                        sb = 4 * g + i
                        acc = psY.tile([128, CSL], F32, tag="y", name="yacc")
                        for kt in range(KT):
                            nc.tensor.matmul(
                                acc[:],
                                og[:, kt * QG + i * 128:
                                   kt * QG + (i + 1) * 128],
                                wos[:, kt * CSL:(kt + 1) * CSL],
                                start=(kt == 0), stop=(kt == KT - 1))
                        yst = ystp.tile([128, CSL], F32, tag="ys", name="yst")
                        nc.vector.tensor_add(yst[:], acc[:], bos[:])
                        nc.sync.dma_start(
                            y_t.ap()[sb * 128:(sb + 1) * 128, :], yst[:])

    nc.compile()
    return nc


def _tilize(w):
    """[E, cols] -> [128, KT*cols]: k-tile kt at columns kt*cols."""
    cols = w.shape[1]
    return np.ascontiguousarray(
        w.reshape(KT, 128, cols).transpose(1, 0, 2).reshape(128, KT * cols))


def _prep_inputs(x, Wq, bq, Wk, bk, Wv, bv, WO, bo):
    import ml_dtypes

    f32 = np.float32
    bf16 = ml_dtypes.bfloat16
    xT = np.ascontiguousarray(np.asarray(x, f32)[0].T).astype(bf16)
    Wq = np.asarray(Wq, f32); Wk = np.asarray(Wk, f32); Wv = np.asarray(Wv, f32)
    bq = np.asarray(bq, f32); bk = np.asarray(bk, f32); bv = np.asarray(bv, f32)
    WO = np.asarray(WO, f32); bo = np.asarray(bo, f32)

    jm = np.arange(4)[:, None, None]
    r = np.arange(128)[None, :, None]
    c = np.arange(QG)[None, None, :]
    mask = (128 * jm + r <= c).astype(bf16).reshape(4 * 128, QG)

    in_maps = []
    for cidx in range(N_CORES):
        h0, h1 = HPC * cidx, HPC * cidx + 1
        in_maps.append({
            "xT": xT,
            "wq": _tilize(np.concatenate([Wq[h0], Wq[h1]], 1)).astype(bf16),
            "wk": _tilize(np.concatenate([Wk[h0], Wk[h1]], 1)).astype(bf16),
            "wv": _tilize(np.concatenate([Wv[h0], Wv[h1]], 1)).astype(bf16),
            "bq": np.ascontiguousarray(np.stack([bq[h0], bq[h1]], 1)),
            "bk": np.ascontiguousarray(np.stack([bk[h0], bk[h1]], 1)),
            "bv": np.concatenate([bv[h0], bv[h1]])[None, :].copy(),
            "wo": _tilize(np.ascontiguousarray(
                WO[:, CSL * cidx:CSL * (cidx + 1)])).astype(bf16),
            "bo": bo[CSL * cidx:CSL * (cidx + 1)][None, :].copy(),
            "mask": mask,
        })
    return in_maps


def kernel(x, Wq, bq, Wk, bk, Wv, bv, WO, bo, trace=False, fp_name="float32r"):
    from concourse.bass_utils import run_bass_kernel_spmd

    key = fp_name
    if key not in _CACHE:
        _CACHE[key] = _build(fp_name)
    nc = _CACHE[key]

    in_maps = _prep_inputs(x, Wq, bq, Wk, bk, Wv, bv, WO, bo)
    kwargs = {}
    if trace:
        kwargs["trace"] = True
    res = run_bass_kernel_spmd(nc, in_maps, core_ids=list(range(N_CORES)),
                               **kwargs)
    kernel.last_results = res

    y = np.concatenate([res.results[c]["y"] for c in range(N_CORES)], axis=1)
    return y.reshape(B, S, E).astype(np.float32)


# revision 17
# speedup vs baseline: 1.0889x; 1.0889x over previous
"""Multi-head causal attention (B=1, S=2048, E=2048, H=16, DH=128) on 8 TRN2
NeuronCores.

Sharding: tensor-parallel over heads; core c owns heads 2c and 2c+1.
Pipeline, ordered so the serial AllGather chain starts as early as possible:
  S1: stream x^T (bf16) while computing Q^T/K^T head 0 (8 PSUM accumulators)
  S2: Q^T/K^T head 1
  S3: V s-blocks 0-7 (both heads)
  attn(0), attn(1)     -> their gathers fire at ~90-110us
  S4: V s-blocks 8-15
  attn(2), attn(3)
  tail: output projection per q-group (column-sharded: core c computes
        y[:, 256c:256(c+1)]); gathers for groups 0-2 are long done, only
        group 3's is partially exposed.
Host gathers by concatenating the 8 column slices.

attention g (both heads, sk-blocks in pairs, software-pipelined emission):
S^T = K @ Q^T, exp on ScalarE, block-causal mask as post-exp multiply,
denominators via a ones-column matmul, normalization via a K=1 broadcast
matmul + DVE multiply.

Precision: x^T and Wq/Wk/Wv bf16 (projection inputs only); Q^T/K^T/V and
all attention matmuls float32r (full PE rate, ~1.2e-4); O^T gather and
output projection bf16; all accumulation fp32 in PSUM.
"""
import os
import sys

if "/opt/trn_rl_repo" not in sys.path:
    sys.path.insert(0, "/opt/trn_rl_repo")

import numpy as np

B, S, E, H = 1, 2048, 2048, 16
DH = E // H          # 128
N_CORES = 8
HPC = H // N_CORES   # heads per core = 2
KT = E // 128        # 16 contraction tiles
QG = 512             # q-group width
NG = S // QG         # 4 q-groups
SBK = S // 128       # 16 s/sk blocks
CSL = E // N_CORES   # 256 output columns per core

_CACHE = {}


def _build(fp_name: str):
    import concourse.bass as bass  # noqa: F401
    import concourse.mybir as mybir
    import concourse.tile as tile
    from concourse import bacc

    FP = getattr(mybir.dt, fp_name)
    F32 = mybir.dt.float32
    BF16 = mybir.dt.bfloat16
    AF = mybir.ActivationFunctionType

    nc = bacc.Bacc("TRN2", target_bir_lowering=False, debug=False,
                   num_devices=N_CORES)

    xT_t = nc.dram_tensor("xT", [E, S], BF16, kind="ExternalInput")
    wq_t = nc.dram_tensor("wq", [128, KT * HPC * DH], BF16, kind="ExternalInput")
    wk_t = nc.dram_tensor("wk", [128, KT * HPC * DH], BF16, kind="ExternalInput")
    wv_t = nc.dram_tensor("wv", [128, KT * HPC * DH], BF16, kind="ExternalInput")
    bq_t = nc.dram_tensor("bq", [DH, HPC], F32, kind="ExternalInput")
    bk_t = nc.dram_tensor("bk", [DH, HPC], F32, kind="ExternalInput")
    bv_t = nc.dram_tensor("bv", [1, HPC * DH], F32, kind="ExternalInput")
    wo_t = nc.dram_tensor("wo", [128, KT * CSL], BF16, kind="ExternalInput")
    bo_t = nc.dram_tensor("bo", [1, CSL], F32, kind="ExternalInput")
    mask_t = nc.dram_tensor("mask", [4 * 128, QG], BF16, kind="ExternalInput")
    y_t = nc.dram_tensor("y", [S, CSL], F32, kind="ExternalOutput")

    xT_r = xT_t.ap().rearrange("(kt p) s -> kt p s", p=128)
    mask_r = mask_t.ap().rearrange("(jm p) q -> jm p q", p=128)

    scale = 1.0 / float(np.sqrt(DH))

    with tile.TileContext(nc) as tc:
        with tc.tile_pool(name="const", bufs=1) as constp, \
             tc.tile_pool(name="prod", bufs=1) as prodp, \
             tc.tile_pool(name="dram", bufs=1, space="DRAM") as dramp:
            # head-0 Q/K weights first: they gate the first matmul
            wqk_sb = {}
            for nm_, t_ in (("wq", wq_t), ("wk", wk_t)):
                for hh in range(HPC):
                    wt = constp.tile([128, KT * DH], BF16,
                                     tag=f"w_{nm_}{hh}", name=f"w_{nm_}{hh}")
                    wqk_sb[(nm_, hh)] = wt
            for nm_, t_ in (("wq", wq_t), ("wk", wk_t)):
                nc.sync.dma_start(wqk_sb[(nm_, 0)][:],
                                  t_.ap()[:, 0:KT * DH])
            bqs = constp.tile([DH, HPC], F32)
            nc.sync.dma_start(bqs[:], bq_t.ap()[:])
            bks = constp.tile([DH, HPC], F32)
            nc.sync.dma_start(bks[:], bk_t.ap()[:])
            ones_f32 = constp.tile([128, 128], F32)
            nc.vector.memset(ones_f32[:], 1.0)
            ones_col = constp.tile([128, 1], FP)
            nc.vector.tensor_copy(ones_col[:], ones_f32[:, 0:1])
            ones_row = constp.tile([1, 128], FP)
            nc.vector.tensor_copy(ones_row[:], ones_f32[0:1, :])
            bvs = constp.tile([128, HPC * DH], F32)
            bos = constp.tile([128, CSL], F32)
            masks = constp.tile([128, 4 * QG], BF16)

            # --- products ---
            qkt = prodp.tile([128, HPC * S], FP)   # Q^T, head hh at cols hh*S
            kkt = prodp.tile([128, HPC * S], FP)   # K^T
            vt = prodp.tile([128, SBK * HPC * DH], FP)  # V, s-block sb at sb*256

            cin = [dramp.tile([HPC * DH, QG], BF16, tag=f"cin{g}",
                              name=f"cin{g}") for g in range(NG)]
            cout = [dramp.tile([N_CORES, HPC * DH, QG], BF16,
                               tag=f"cout{g}", name=f"cout{g}",
                               addr_space="Shared") for g in range(NG)]

            wv_sb = constp.tile([128, KT * HPC * DH], BF16, tag="wv_sb",
                                name="wv_sb")

            with tc.tile_pool(name="xt", bufs=1) as xtp, \
                 tc.tile_pool(name="pt", bufs=6) as ptp, \
                 tc.tile_pool(name="osb", bufs=1) as osbp, \
                 tc.tile_pool(name="rec", bufs=2) as recp, \
                 tc.tile_pool(name="bcs", bufs=2) as bcsp:
                xt = xtp.tile([128, KT * S], BF16)
                o_sbuf = osbp.tile([128, HPC * S], BF16)

                def qk_pass(hh, stream):
                    """Q^T and K^T for head hh, all 4 q-groups: 8 PSUM accs."""
                    with tc.tile_pool(name=f"psQK{hh}", bufs=8,
                                      space="PSUM") as psA:
                        specs = [("wq", qkt, bqs), ("wk", kkt, bks)]
                        accs = {p: [psA.tile([128, QG], F32, tag="qk",
                                             name=f"qk{hh}_{p}{g}")
                                    for g in range(NG)] for p in range(2)}
                        for kt in range(KT):
                            if stream:
                                nc.sync.dma_start(xt[:, kt * S:(kt + 1) * S],
                                                  xT_r[kt])
                            for p, (wn, prod, bias) in enumerate(specs):
                                wtile = wqk_sb[(wn, hh)][:, kt * DH:
                                                         (kt + 1) * DH]
                                for g in range(NG):
                                    nc.tensor.matmul(
                                        accs[p][g][:], wtile,
                                        xt[:, kt * S + g * QG:
                                           kt * S + (g + 1) * QG],
                                        start=(kt == 0), stop=(kt == KT - 1))
                        for p, (wn, prod, bias) in enumerate(specs):
                            for g in range(NG):
                                nc.scalar.activation(
                                    prod[:, hh * S + g * QG:
                                         hh * S + (g + 1) * QG],
                                    accs[p][g][:], AF.Identity,
                                    bias=bias[:, hh:hh + 1])

                def v_pass(half):
                    """V s-blocks half*8 .. half*8+7, both heads."""
                    with tc.tile_pool(name=f"psV{half}", bufs=8,
                                      space="PSUM") as psV:
                        accs = [psV.tile([128, HPC * DH], F32, tag="v",
                                         name=f"v{half}_{i}")
                                for i in range(8)]
                        for kt in range(KT):
                            wvtile = wv_sb[:, kt * HPC * DH:
                                           (kt + 1) * HPC * DH]
                            for i in range(8):
                                sb = half * 8 + i
                                nc.tensor.matmul(
                                    accs[i][:],
                                    xt[:, kt * S + sb * 128:
                                       kt * S + (sb + 1) * 128],
                                    wvtile,
                                    start=(kt == 0), stop=(kt == KT - 1))
                        for i in range(8):
                            sb = half * 8 + i
                            nc.vector.tensor_add(
                                vt[:, sb * HPC * DH:(sb + 1) * HPC * DH],
                                accs[i][:], bvs[:])

                def attn(g):
                    npairs = 2 * g + 2
                    jmax = 4 * g + 3
                    with tc.tile_pool(name=f"psS{g}", bufs=2,
                                      space="PSUM") as psS, \
                         tc.tile_pool(name=f"psO{g}", bufs=2,
                                      space="PSUM") as psO, \
                         tc.tile_pool(name=f"psN{g}", bufs=2,
                                      space="PSUM") as psN:
                        o_acc = [psO.tile([128, QG], F32, tag="o",
                                          name=f"o{hh}") for hh in range(HPC)]
                        s_acc = [psN.tile([1, QG], F32, tag="n",
                                          name=f"n{hh}") for hh in range(HPC)]

                        def emit_pv(hh, jp, pt):
                            for dj in range(2):
                                j = 2 * jp + dj
                                nc.tensor.matmul(
                                    o_acc[hh][:],
                                    vt[:, j * HPC * DH + hh * DH:
                                       j * HPC * DH + (hh + 1) * DH],
                                    pt[:, dj * QG:(dj + 1) * QG],
                                    start=(j == 0), stop=(j == jmax))
                                nc.tensor.matmul(
                                    s_acc[hh][:], ones_col[:],
                                    pt[:, dj * QG:(dj + 1) * QG],
                                    start=(j == 0), stop=(j == jmax))

                        pend = []
                        for jp in range(npairs):
                            for hh in range(HPC):
                                ps = psS.tile([128, 2 * QG], F32, tag="s",
                                              name="ps")
                                for dj in range(2):
                                    j = 2 * jp + dj
                                    nc.tensor.matmul(
                                        ps[:, dj * QG:(dj + 1) * QG],
                                        kkt[:, hh * S + j * 128:
                                            hh * S + (j + 1) * 128],
                                        qkt[:, hh * S + g * QG:
                                            hh * S + (g + 1) * QG],
                                        start=True, stop=True)
                                pt = ptp.tile([128, 2 * QG], FP, tag="p",
                                              name="pt")
                                nc.scalar.activation(pt[:], ps[:], AF.Exp,
                                                     scale=scale)
                                if 2 * jp >= 4 * g:
                                    jms = 2 * jp - 4 * g
                                    nc.vector.tensor_mul(
                                        pt[:], pt[:],
                                        masks[:, jms * QG:(jms + 2) * QG])
                                pend.append((hh, jp, pt))
                                while len(pend) > 2:
                                    emit_pv(*pend.pop(0))
                        while pend:
                            emit_pv(*pend.pop(0))

                        for hh in range(HPC):
                            rec = recp.tile([1, QG], FP, tag="r", name="rec")
                            with nc.allow_low_precision(
                                    reason="softmax denom recip in fp32r"):
                                nc.vector.reciprocal(rec[:], s_acc[hh][:])
                            bc = psS.tile([128, QG], F32, tag="s", name="bc")
                            nc.tensor.matmul(bc[:], ones_row[:], rec[:],
                                             start=True, stop=True)
                            bcs = bcsp.tile([128, QG], F32, tag="b",
                                            name="bcs")
                            nc.vector.tensor_copy(bcs[:], bc[:])
                            nc.vector.tensor_mul(
                                o_sbuf[:, hh * S + g * QG:
                                       hh * S + (g + 1) * QG],
                                o_acc[hh][:], bcs[:])
                            nc.sync.dma_start(
                                cin[g].rearrange("(hh p) q -> hh p q",
                                                 p=128)[hh],
                                o_sbuf[:, hh * S + g * QG:
                                       hh * S + (g + 1) * QG])
                    nc.gpsimd.collective_compute(
                        "AllGather",
                        mybir.AluOpType.bypass,
                        replica_groups=[list(range(N_CORES))],
                        ins=[cin[g].opt()],
                        outs=[cout[g].opt()],
                    )

                qk_pass(0, stream=True)
                for nm_, t_ in (("wq", wq_t), ("wk", wk_t)):
                    nc.sync.dma_start(wqk_sb[(nm_, 1)][:],
                                      t_.ap()[:, KT * DH:2 * KT * DH])
                nc.sync.dma_start(wv_sb[:], wv_t.ap()[:])
                nc.sync.dma_start(bvs[:],
                                  bv_t.ap().to_broadcast((128, HPC * DH)))
                for jm in range(4):
                    nc.sync.dma_start(masks[:, jm * QG:(jm + 1) * QG],
                                      mask_r[jm])
                qk_pass(1, stream=False)
                v_pass(0)
                attn(0)
                attn(1)
                v_pass(1)
                attn(2)
                attn(3)
                nc.sync.dma_start(bos[:], bo_t.ap().to_broadcast((128, CSL)))

            # ===== tail: output projection, column-sharded =====
            with tc.tile_pool(name="wo", bufs=1) as wop, \
                 tc.tile_pool(name="og", bufs=2) as ogp, \
                 tc.tile_pool(name="yst", bufs=2) as ystp, \
                 tc.tile_pool(name="psY", bufs=4, space="PSUM") as psY:
                wos = wop.tile([128, KT * CSL], BF16)
                nc.sync.dma_start(wos[:], wo_t.ap()[:])
                for g in range(NG):
                    og = ogp.tile([128, KT * QG], BF16, tag="og", name="og")
                    for kt in range(KT):
                        nc.sync.dma_start(
                            og[:, kt * QG:(kt + 1) * QG],
                            cout[g][kt // HPC, (kt % HPC) * 128:
                                    (kt % HPC) * 128 + 128, :])
                    for i in range(4):
                        sb = 4 * g + i
                        acc = psY.tile([128, CSL], F32, tag="y", name="yacc")
                        for kt in range(KT):
                            nc.tensor.matmul(
                                acc[:],
                                og[:, kt * QG + i * 128:
                                   kt * QG + (i + 1) * 128],
                                wos[:, kt * CSL:(kt + 1) * CSL],
                                start=(kt == 0), stop=(kt == KT - 1))
                        yst = ystp.tile([128, CSL], F32, tag="ys", name="yst")
                        nc.vector.tensor_add(yst[:], acc[:], bos[:])
                        nc.sync.dma_start(
                            y_t.ap()[sb * 128:(sb + 1) * 128, :], yst[:])

    nc.compile()
    return nc


def _tilize(w):
    """[E, cols] -> [128, KT*cols]: k-tile kt at columns kt*cols."""
    cols = w.shape[1]
    return np.ascontiguousarray(
        w.reshape(KT, 128, cols).transpose(1, 0, 2).reshape(128, KT * cols))


def _tilize_hm(w):
    """[E, HPC*DH] -> [128, HPC*KT*DH], head-major then k-tile."""
    return np.ascontiguousarray(
        w.reshape(KT, 128, HPC, DH).transpose(1, 2, 0, 3)
        .reshape(128, HPC * KT * DH))


def _prep_inputs(x, Wq, bq, Wk, bk, Wv, bv, WO, bo):
    import ml_dtypes

    f32 = np.float32
    bf16 = ml_dtypes.bfloat16
    xT = np.ascontiguousarray(np.asarray(x, f32)[0].T).astype(bf16)
    Wq = np.asarray(Wq, f32); Wk = np.asarray(Wk, f32); Wv = np.asarray(Wv, f32)
    bq = np.asarray(bq, f32); bk = np.asarray(bk, f32); bv = np.asarray(bv, f32)
    WO = np.asarray(WO, f32); bo = np.asarray(bo, f32)

    jm = np.arange(4)[:, None, None]
    r = np.arange(128)[None, :, None]
    c = np.arange(QG)[None, None, :]
    mask = (128 * jm + r <= c).astype(bf16).reshape(4 * 128, QG)

    in_maps = []
    for cidx in range(N_CORES):
        h0, h1 = HPC * cidx, HPC * cidx + 1
        in_maps.append({
            "xT": xT,
            "wq": _tilize_hm(np.concatenate([Wq[h0], Wq[h1]], 1)).astype(bf16),
            "wk": _tilize_hm(np.concatenate([Wk[h0], Wk[h1]], 1)).astype(bf16),
            "wv": _tilize(np.concatenate([Wv[h0], Wv[h1]], 1)).astype(bf16),
            "bq": np.ascontiguousarray(np.stack([bq[h0], bq[h1]], 1)),
            "bk": np.ascontiguousarray(np.stack([bk[h0], bk[h1]], 1)),
            "bv": np.concatenate([bv[h0], bv[h1]])[None, :].copy(),
            "wo": _tilize(np.ascontiguousarray(
                WO[:, CSL * cidx:CSL * (cidx + 1)])).astype(bf16),
            "bo": bo[CSL * cidx:CSL * (cidx + 1)][None, :].copy(),
            "mask": mask,
        })
    return in_maps


def kernel(x, Wq, bq, Wk, bk, Wv, bv, WO, bo, trace=False, fp_name="float32r"):
    from concourse.bass_utils import run_bass_kernel_spmd

    key = fp_name
    if key not in _CACHE:
        _CACHE[key] = _build(fp_name)
    nc = _CACHE[key]

    in_maps = _prep_inputs(x, Wq, bq, Wk, bk, Wv, bv, WO, bo)
    kwargs = {}
    if trace:
        kwargs["trace"] = True
    res = run_bass_kernel_spmd(nc, in_maps, core_ids=list(range(N_CORES)),
                               **kwargs)
    kernel.last_results = res

    y = np.concatenate([res.results[c]["y"] for c in range(N_CORES)], axis=1)
    return y.reshape(B, S, E).astype(np.float32)
